# revision 2
# baseline (speedup 1.0000x reference)
"""AttentionPooling kernel v6 for Trainium2 (8 NeuronCores, SPMD).

reference math:
    scores = tanh(x @ W1 + b1) @ W2 + b2        # [N, 1]
    attn   = softmax(scores, axis=0)            # global over all N rows
    pooled = segment_sum(x * attn, batch, 1024) # [1024, 256]

v5 (all-bf16). vs v2 baseline:
  - NO PE transposes: the transposed copy of x covers all 16 tiles per
    supertile (DMA has concurrent-queue headroom; PE was the wall).
    Frees 2 PSUM banks -> htp triple-buffered, and kills the DVE
    PSUM->SBUF copies.
  - masks built per block in 2 wide DVE ops instead of 4 per-tile
    tensor_scalar ops.  Layout [p, g, t] keeps the broadcast (stride-0)
    dims off the innermost axis so DVE runs in 2x mode.  The one-hot
    half ((iota==brel), static data) is built 4 blocks early, off the
    exp->mask->pool critical path; only the *e multiply is on it.
  - scores PSUM carved out of the accumulator bank (sub-bank views),
    so PSUM = htp 3x2 + acc/sp 1 + spare.
  - xrow / xT in separate contiguous DRAM tensors, each DMA 1 MB, on the
    two hwdge queues concurrently; xr buffered 6 deep so the pool (its
    last reader) never write-blocks the prefetch DMA.
"""

import numpy as np
import ml_dtypes
from contextlib import ExitStack

import concourse.bass as bass
import concourse.bacc as bacc
import concourse.mybir as mybir
import concourse.tile as tile
from concourse.bass_utils import run_bass_kernel_spmd

F32 = mybir.dt.float32
BF16 = mybir.dt.bfloat16
I32 = mybir.dt.int32
BF = ml_dtypes.bfloat16

NUM_GRAPHS = 1024
NC = 8
GPC = NUM_GRAPHS // NC  # graphs per core = 128
P = 128
D = 256
ST = 16          # tiles per DMA supertile (2048 rows)
BLK = 4          # tiles per block (512 rows)
SW = 64          # score matmul free width (W2 dup count)


def build_program(R: int, T: int, with_b1: bool) -> bass.Bass:
    assert R == T * P and T % ST == 0
    nsup = T // ST
    nblk = T // BLK
    bps = ST // BLK  # blocks per supertile = 4

    nc = bacc.Bacc("TRN2", target_bir_lowering=False, debug=False)
    xrow_d = nc.declare_dram_parameter(
        "xrow", [nsup * P, ST * D], BF16, isOutput=False)
    xt_d = nc.declare_dram_parameter(
        "xt", [nsup * P, 2 * ST * P], BF16, isOutput=False)
    w1_d = nc.declare_dram_parameter("w1", [P, 2 * 2 * P], BF16, isOutput=False)
    w2_d = nc.declare_dram_parameter("w2", [P, 2 * SW], BF16, isOutput=False)
    brel_d = nc.declare_dram_parameter("brel", [P, T], BF16, isOutput=False)
    if with_b1:
        b1_d = nc.declare_dram_parameter("b1d", [P, 2], F32, isOutput=False)
    pooled_d = nc.declare_dram_parameter("pooled", [P, D], F32, isOutput=True)
    evec_d = nc.declare_dram_parameter("evec_out", [P, T], BF16, isOutput=True)

    Tanh = mybir.ActivationFunctionType.Tanh
    Exp = mybir.ActivationFunctionType.Exp

    with ExitStack() as ctx:
        tc = ctx.enter_context(tile.TileContext(nc))
        const = ctx.enter_context(tc.tile_pool(name="const", bufs=1))
        xrp = ctx.enter_context(tc.tile_pool(name="xr", bufs=6))
        xtp = ctx.enter_context(tc.tile_pool(name="xt", bufs=4))
        htpp = ctx.enter_context(tc.tile_pool(name="htp", bufs=3, space="PSUM"))
        thp = ctx.enter_context(tc.tile_pool(name="th", bufs=3))
        ohp = ctx.enter_context(tc.tile_pool(name="oh", bufs=6))
        mkp = ctx.enter_context(tc.tile_pool(name="mk", bufs=3))
        spp = ctx.enter_context(tc.tile_pool(name="sp", bufs=1, space="PSUM"))
        accp = ctx.enter_context(tc.tile_pool(name="acc", bufs=1, space="PSUM"))
        outp = ctx.enter_context(tc.tile_pool(name="out", bufs=1))

        # ---- constants ----
        w1sb = const.tile([P, 2, 2, P], BF16, tag="w1sb")  # [d_lo, dc, jc, j_lo]
        nc.sync.dma_start(w1sb[:], w1_d.rearrange("p (a b j) -> p a b j", a=2, b=2))
        w2sb = const.tile([P, 2, SW], BF16, tag="w2sb")  # [j_lo, jc, dup]
        nc.scalar.dma_start(w2sb[:], w2_d.rearrange("p (a r) -> p a r", a=2))
        brelsb = const.tile([P, T], BF16, tag="brelsb")
        nc.sync.dma_start(brelsb[:], brel_d[:])
        if with_b1:
            b1sb = const.tile([P, 2], F32, tag="b1sb")
            nc.scalar.dma_start(b1sb[:], b1_d[:])

        iota_i = const.tile([P, P], I32)
        nc.gpsimd.iota(iota_i[:], pattern=[[1, P]], base=0, channel_multiplier=0)
        iota_p = const.tile([P, P], BF16, tag="iota_p")
        nc.vector.tensor_copy(iota_p[:], iota_i[:])
        # iota_gt[p, g, t] = g  (materialized so mask builds stay 2x-packed)
        iota_gt = const.tile([P, P, BLK], BF16, tag="iota_gt")
        nc.vector.tensor_copy(
            iota_gt[:], iota_p[:].unsqueeze(2).broadcast_to([P, P, BLK]))

        warm = const.tile([P, 2], F32, tag="warm")
        nc.gpsimd.memset(warm[:], 0.0)
        nc.scalar.activation(warm[:], warm[:], Tanh)  # pull ACT table early

        evec = const.tile([P, T], BF16, tag="evec")

        acc = accp.tile([P, D], F32)  # pooled[g, d], persistent PSUM bank

        xr_t = {}
        xt_t = {}
        htp_t = {}
        th_t = {}
        oh_t = {}
        mk_t = {}

        def issue_sup_dma(s):
            xr_t[s] = xrp.tile([P, ST, D], BF16, tag="xr", name=f"xr{s}")
            nc.sync.dma_start(
                xr_t[s][:],
                xrow_d[s * P:(s + 1) * P, :].rearrange(
                    "p (t d) -> p t d", t=ST),
            )
            xt_t[s] = xtp.tile([P, 2, ST, P], BF16, tag="xt", name=f"xt{s}")
            nc.scalar.dma_start(
                xt_t[s][:],
                xt_d[s * P:(s + 1) * P, :].rearrange(
                    "p (a t d) -> p a t d", a=2, t=ST),
            )

        # supertile 0's xT arrives as 4 block-chunks on both queues so the
        # first hT can start ~3us earlier
        xr_t[0] = xrp.tile([P, ST, D], BF16, tag="xr", name="xr0")
        nc.sync.dma_start(
            xr_t[0][:],
            xrow_d[0:P, :].rearrange("p (t d) -> p t d", t=ST))
        xt_t[0] = xtp.tile([P, 2, ST, P], BF16, tag="xt", name="xt0")
        for bis in range(bps):
            eng = nc.scalar if bis % 2 == 0 else nc.sync
            for dc in range(2):
                eng.dma_start(
                    xt_t[0][:, dc, bis * BLK:(bis + 1) * BLK, :],
                    xt_d[0:P, :].rearrange(
                        "p (a t d) -> p a t d", a=2, t=ST
                    )[:, dc, bis * BLK:(bis + 1) * BLK, :])
        for s in range(1, min(4, nsup)):
            issue_sup_dma(s)

        for g in range(nblk + 8):
            # ---- stage A: DMA prefetch + hT GEMM + one-hot for block g ----
            if g < nblk:
                s, bis = divmod(g, bps)
                if bis == 0 and s + 4 < nsup:
                    issue_sup_dma(s + 4)
                htp_t[g] = htpp.tile([P, 2, BLK * P], F32, tag="htp",
                                     name=f"htp{g}")
                for jc in range(2):
                    for dc in range(2):
                        nc.tensor.matmul(
                            htp_t[g][:, jc, :],
                            lhsT=w1sb[:, dc, jc, :],
                            rhs=xt_t[s][:, dc, bis * BLK:(bis + 1) * BLK, :],
                            start=(dc == 0),
                            stop=(dc == 1),
                        )
                # one-hot (static): oh[p, g, t] = (g == brel[p, blk*4+t])
                oh_t[g] = ohp.tile([P, P, BLK], BF16, tag="oh", name=f"oh{g}")
                brel_b = brelsb[:, g * BLK:(g + 1) * BLK].unsqueeze(
                    1).broadcast_to([P, P, BLK])
                nc.vector.tensor_tensor(
                    oh_t[g][:], iota_gt[:], brel_b,
                    op=mybir.AluOpType.is_equal)

            # ---- stage B: tanh for block g-1 ----
            b = g - 1
            if 0 <= b < nblk:
                th_t[b] = thp.tile([P, 2, BLK * P], BF16, tag="th",
                                   name=f"th{b}")
                if with_b1:
                    for jc in range(2):
                        nc.scalar.activation(
                            th_t[b][:, jc], htp_t[b][:, jc], Tanh,
                            bias=b1sb[:, jc:jc + 1],
                        )
                else:
                    nc.scalar.activation(th_t[b][:], htp_t[b][:], Tanh)
                del htp_t[b]

            # ---- stage C: score matmuls + exp for block g-2 ----
            b = g - 2
            if 0 <= b < nblk:
                sp = spp.tile([P, BLK, SW], F32, tag="sp", name=f"sp{b}")
                for tt in range(BLK):
                    for jc in range(2):
                        nc.tensor.matmul(
                            sp[:, tt, :],
                            lhsT=th_t[b][:, jc, tt * P:(tt + 1) * P],
                            rhs=w2sb[:, jc, :],
                            start=(jc == 0),
                            stop=(jc == 1),
                            skip_group_check=True,
                        )
                c0 = b * BLK
                nc.scalar.activation(evec[:, c0:c0 + BLK], sp[:, :, 0:1], Exp)
                del th_t[b]

            # ---- stage D: mask multiply for block g-3 ----
            b = g - 3
            if 0 <= b < nblk:
                c0 = b * BLK
                mk_t[b] = mkp.tile([P, P, BLK], BF16, tag="mk", name=f"mk{b}")
                ev_b = evec[:, c0:c0 + BLK].unsqueeze(1).broadcast_to(
                    [P, P, BLK])
                nc.vector.tensor_tensor(
                    mk_t[b][:], oh_t[b][:], ev_b, op=mybir.AluOpType.mult)
                del oh_t[b]

            # ---- stage E: pool matmuls for block g-4 ----
            b = g - 4
            if 0 <= b < nblk:
                s, bis = divmod(b, bps)
                for tt in range(BLK):
                    tg = b * BLK + tt
                    nc.tensor.matmul(
                        acc,
                        lhsT=mk_t[b][:, :, tt],
                        rhs=xr_t[s][:, bis * BLK + tt, :],
                        start=(tg == 0),
                        stop=(tg == T - 1),
                        skip_group_check=True,
                    )
                del mk_t[b]
                if bis == bps - 1 and s > 0:
                    xr_t.pop(s - 1, None)

        out_sb = outp.tile([P, D], F32)
        nc.vector.tensor_copy(out_sb[:], acc)
        nc.sync.dma_start(pooled_d[:], out_sb[:])
        nc.scalar.dma_start(evec_d[:], evec[:])

    nc.compile()
    return nc


def _prep_inputs(x, batch, W1, b1, W2):
    """Shard rows at graph boundaries; build bf16 layouts."""
    x16 = np.asarray(x, dtype=np.float32).astype(BF)
    batch = np.asarray(batch)
    bounds = np.searchsorted(batch, np.arange(0, NUM_GRAPHS + 1, GPC))
    counts = np.diff(bounds)
    chunk = ST * P
    R = int(np.ceil(max(int(counts.max()), 1) / chunk) * chunk)
    T = R // P
    nsup = T // ST

    b1h = np.asarray(b1, dtype=np.float32).reshape(-1)
    with_b1 = bool(np.any(b1h))
    w1h = np.ascontiguousarray(
        np.asarray(W1, dtype=np.float32).reshape(2, P, 2, P).transpose(1, 0, 2, 3)
    ).astype(BF).reshape(P, 2 * 2 * P)
    w2h = np.repeat(
        np.asarray(W2, dtype=np.float32).reshape(2, P).transpose(1, 0)[:, :, None],
        64, axis=2,
    ).astype(BF).reshape(P, 2 * 64)
    b1_pt = np.ascontiguousarray(b1h.reshape(2, P).transpose(1, 0))

    in_maps = []
    for c in range(NC):
        lo, hi = int(bounds[c]), int(bounds[c + 1])
        n = hi - lo
        xs = np.zeros((R, D), dtype=BF)
        xs[:n] = x16[lo:hi]
        x5 = xs.reshape(nsup, ST, P, 2, P)
        # xrow[s*128+p, (t, d)] = x[s*2048+t*128+p, d]
        xr_h = np.ascontiguousarray(
            x5.transpose(0, 2, 1, 3, 4)
        ).reshape(nsup * P, ST * D)
        # xT[s*128+d_lo, (dc, t, i_lo)] = x[s*2048+t*128+i, dc*128+d_lo]
        xt_h = np.ascontiguousarray(
            x5.transpose(0, 4, 3, 1, 2)
        ).reshape(nsup * P, 2 * ST * P)

        br = np.full((R,), -1.0, dtype=np.float32)
        br[:n] = (np.asarray(batch[lo:hi], dtype=np.int64) - c * GPC).astype(
            np.float32)
        brel_pt = np.ascontiguousarray(
            br.reshape(T, P).transpose(1, 0)).astype(BF)
        m = {"xrow": xr_h, "xt": xt_h, "w1": w1h, "w2": w2h, "brel": brel_pt}
        if with_b1:
            m["b1d"] = b1_pt
        in_maps.append(m)
    return in_maps, R, T, with_b1, [int(c) for c in counts]


def run(x, batch, W1, b1, W2, b2, trace=False, trace_kwargs=None):
    in_maps, R, T, with_b1, counts = _prep_inputs(x, batch, W1, b1, W2)
    nc = build_program(R, T, with_b1)
    res = run_bass_kernel_spmd(
        nc, in_maps, core_ids=list(range(NC)), trace=trace,
        **(trace_kwargs or {}),
    )
    pooled = np.zeros((NUM_GRAPHS, D), dtype=np.float64)
    Z = 0.0
    for c in range(NC):
        pooled[c * GPC:(c + 1) * GPC, :] = (
            res.results[c]["pooled"].astype(np.float64))
        ev = res.results[c]["evec_out"].astype(np.float64)  # [P, T] bf16
        n = counts[c]
        rows = ev.transpose(1, 0).reshape(-1)  # row r = t*128+p order
        Z += rows[:n].sum()
    out = (pooled / Z).astype(np.float32)
    return out, res


def kernel(x, batch, W1, b1, W2, b2):
    out, _ = run(x, batch, W1, b1, W2, b2)
    return out


# revision 3
# speedup vs baseline: 1.0277x; 1.0277x over previous
"""AttentionPooling kernel v7 for Trainium2 (8 NeuronCores, SPMD).

reference math:
    scores = tanh(x @ W1 + b1) @ W2 + b2        # [N, 1]
    attn   = softmax(scores, axis=0)            # global over all N rows
    pooled = segment_sum(x * attn, batch, 1024) # [1024, 256]

v5 (all-bf16). vs v2 baseline:
  - NO PE transposes: the transposed copy of x covers all 16 tiles per
    supertile (DMA has concurrent-queue headroom; PE was the wall).
    Frees 2 PSUM banks -> htp triple-buffered, and kills the DVE
    PSUM->SBUF copies.
  - masks built per block in 2 wide DVE ops instead of 4 per-tile
    tensor_scalar ops.  Layout [p, g, t] keeps the broadcast (stride-0)
    dims off the innermost axis so DVE runs in 2x mode.  The one-hot
    half ((iota==brel), static data) is built 4 blocks early, off the
    exp->mask->pool critical path; only the *e multiply is on it.
  - scores PSUM carved out of the accumulator bank (sub-bank views),
    so PSUM = htp 3x2 + acc/sp 1 + spare.
  - xrow / xT in separate contiguous DRAM tensors, each DMA 1 MB, on the
    two hwdge queues concurrently; xr buffered 6 deep so the pool (its
    last reader) never write-blocks the prefetch DMA.
"""

import numpy as np
import ml_dtypes
from contextlib import ExitStack

import concourse.bass as bass
import concourse.bacc as bacc
import concourse.mybir as mybir
import concourse.tile as tile
from concourse.bass_utils import run_bass_kernel_spmd

F32 = mybir.dt.float32
BF16 = mybir.dt.bfloat16
I32 = mybir.dt.int32
BF = ml_dtypes.bfloat16

NUM_GRAPHS = 1024
NC = 8
GPC = NUM_GRAPHS // NC  # graphs per core = 128
P = 128
D = 256
ST = 16          # tiles per DMA supertile (2048 rows)
BLK = 4          # tiles per block (512 rows)
SW = 16          # score matmul free width (W2 dup count)


def build_program(R: int, T: int, with_b1: bool) -> bass.Bass:
    assert R == T * P and T % ST == 0
    nsup = T // ST
    nblk = T // BLK
    bps = ST // BLK  # blocks per supertile = 4

    nc = bacc.Bacc("TRN2", target_bir_lowering=False, debug=False)
    xrow_d = nc.declare_dram_parameter(
        "xrow", [nsup * P, ST * D], BF16, isOutput=False)
    xt_d = nc.declare_dram_parameter(
        "xt", [nsup * P, 2 * ST * P], BF16, isOutput=False)
    w1_d = nc.declare_dram_parameter("w1", [P, 2 * 2 * P], BF16, isOutput=False)
    w2_d = nc.declare_dram_parameter("w2", [P, 2 * SW], BF16, isOutput=False)
    brel_d = nc.declare_dram_parameter("brel", [P, T], BF16, isOutput=False)
    if with_b1:
        b1_d = nc.declare_dram_parameter("b1d", [P, 2], F32, isOutput=False)
    pooled_d = nc.declare_dram_parameter("pooled", [P, D], F32, isOutput=True)
    evec_d = nc.declare_dram_parameter("evec_out", [P, T], BF16, isOutput=True)

    Tanh = mybir.ActivationFunctionType.Tanh
    Exp = mybir.ActivationFunctionType.Exp

    with ExitStack() as ctx:
        tc = ctx.enter_context(tile.TileContext(nc))
        const = ctx.enter_context(tc.tile_pool(name="const", bufs=1))
        xrp = ctx.enter_context(tc.tile_pool(name="xr", bufs=6))
        xtp = ctx.enter_context(tc.tile_pool(name="xt", bufs=4))
        htpp = ctx.enter_context(tc.tile_pool(name="htp", bufs=3, space="PSUM"))
        thp = ctx.enter_context(tc.tile_pool(name="th", bufs=3))
        ohp = ctx.enter_context(tc.tile_pool(name="oh", bufs=6))
        mkp = ctx.enter_context(tc.tile_pool(name="mk", bufs=3))
        spp = ctx.enter_context(tc.tile_pool(name="sp", bufs=1, space="PSUM"))
        accp = ctx.enter_context(tc.tile_pool(name="acc", bufs=1, space="PSUM"))
        outp = ctx.enter_context(tc.tile_pool(name="out", bufs=1))

        # ---- constants ----
        w1sb = const.tile([P, 2, 2, P], BF16, tag="w1sb")  # [d_lo, dc, jc, j_lo]
        nc.sync.dma_start(w1sb[:], w1_d.rearrange("p (a b j) -> p a b j", a=2, b=2))
        w2sb = const.tile([P, 2, SW], BF16, tag="w2sb")  # [j_lo, jc, dup]
        nc.scalar.dma_start(w2sb[:], w2_d.rearrange("p (a r) -> p a r", a=2))
        brelsb = const.tile([P, T], BF16, tag="brelsb")
        nc.sync.dma_start(brelsb[:], brel_d[:])
        if with_b1:
            b1sb = const.tile([P, 2], F32, tag="b1sb")
            nc.scalar.dma_start(b1sb[:], b1_d[:])

        iota_i = const.tile([P, P], I32)
        nc.gpsimd.iota(iota_i[:], pattern=[[1, P]], base=0, channel_multiplier=0)
        iota_p = const.tile([P, P], BF16, tag="iota_p")
        nc.vector.tensor_copy(iota_p[:], iota_i[:])
        # iota_gt[p, g, t] = g  (materialized so mask builds stay 2x-packed)
        iota_gt = const.tile([P, P, BLK], BF16, tag="iota_gt")
        nc.vector.tensor_copy(
            iota_gt[:], iota_p[:].unsqueeze(2).broadcast_to([P, P, BLK]))

        warm = const.tile([P, 2], F32, tag="warm")
        nc.gpsimd.memset(warm[:], 0.0)
        nc.scalar.activation(warm[:], warm[:], Tanh)  # pull ACT table early

        evec = const.tile([P, T], BF16, tag="evec")

        acc = accp.tile([P, D], F32)  # pooled[g, d], persistent PSUM bank

        xr_t = {}
        xt_t = {}
        htp_t = {}
        th_t = {}
        oh_t = {}
        mk_t = {}

        def issue_sup_dma(s):
            xr_t[s] = xrp.tile([P, ST, D], BF16, tag="xr", name=f"xr{s}")
            nc.sync.dma_start(
                xr_t[s][:],
                xrow_d[s * P:(s + 1) * P, :].rearrange(
                    "p (t d) -> p t d", t=ST),
            )
            xt_t[s] = xtp.tile([P, 2, ST, P], BF16, tag="xt", name=f"xt{s}")
            nc.scalar.dma_start(
                xt_t[s][:],
                xt_d[s * P:(s + 1) * P, :].rearrange(
                    "p (a t d) -> p a t d", a=2, t=ST),
            )

        # supertile 0's xT arrives as 4 block-chunks on both queues so the
        # first hT can start ~3us earlier
        xr_t[0] = xrp.tile([P, ST, D], BF16, tag="xr", name="xr0")
        nc.sync.dma_start(
            xr_t[0][:],
            xrow_d[0:P, :].rearrange("p (t d) -> p t d", t=ST))
        xt_t[0] = xtp.tile([P, 2, ST, P], BF16, tag="xt", name="xt0")
        for bis in range(bps):
            eng = nc.scalar if bis % 2 == 0 else nc.sync
            for dc in range(2):
                eng.dma_start(
                    xt_t[0][:, dc, bis * BLK:(bis + 1) * BLK, :],
                    xt_d[0:P, :].rearrange(
                        "p (a t d) -> p a t d", a=2, t=ST
                    )[:, dc, bis * BLK:(bis + 1) * BLK, :])
        for s in range(1, min(4, nsup)):
            issue_sup_dma(s)

        for g in range(nblk + 8):
            # ---- stage E: pool matmuls for block g-4 (always-ready work
            # first: insulates the PE FIFO from DMA/ACT-dependent stalls) --
            b = g - 4
            if 0 <= b < nblk:
                s, bis = divmod(b, bps)
                for tt in range(BLK):
                    tg = b * BLK + tt
                    nc.tensor.matmul(
                        acc,
                        lhsT=mk_t[b][:, :, tt],
                        rhs=xr_t[s][:, bis * BLK + tt, :],
                        start=(tg == 0),
                        stop=(tg == T - 1),
                        skip_group_check=True,
                    )
                del mk_t[b]
                if bis == bps - 1 and s > 0:
                    xr_t.pop(s - 1, None)

            # ---- stage A: DMA prefetch + hT GEMM + one-hot for block g ----
            if g < nblk:
                s, bis = divmod(g, bps)
                if bis == 0 and s + 4 < nsup:
                    issue_sup_dma(s + 4)
                htp_t[g] = htpp.tile([P, 2, BLK * P], F32, tag="htp",
                                     name=f"htp{g}")
                for jc in range(2):
                    for dc in range(2):
                        nc.tensor.matmul(
                            htp_t[g][:, jc, :],
                            lhsT=w1sb[:, dc, jc, :],
                            rhs=xt_t[s][:, dc, bis * BLK:(bis + 1) * BLK, :],
                            start=(dc == 0),
                            stop=(dc == 1),
                        )
                # one-hot (static): oh[p, g, t] = (g == brel[p, blk*4+t])
                oh_t[g] = ohp.tile([P, P, BLK], BF16, tag="oh", name=f"oh{g}")
                brel_b = brelsb[:, g * BLK:(g + 1) * BLK].unsqueeze(
                    1).broadcast_to([P, P, BLK])
                nc.vector.tensor_tensor(
                    oh_t[g][:], iota_gt[:], brel_b,
                    op=mybir.AluOpType.is_equal)

            # ---- stage B: tanh for block g-1 ----
            b = g - 1
            if 0 <= b < nblk:
                th_t[b] = thp.tile([P, 2, BLK * P], BF16, tag="th",
                                   name=f"th{b}")
                if with_b1:
                    for jc in range(2):
                        nc.scalar.activation(
                            th_t[b][:, jc], htp_t[b][:, jc], Tanh,
                            bias=b1sb[:, jc:jc + 1],
                        )
                else:
                    nc.scalar.activation(th_t[b][:], htp_t[b][:], Tanh)
                del htp_t[b]

            # ---- stage C: score matmuls + exp for block g-2 ----
            b = g - 2
            if 0 <= b < nblk:
                sp = spp.tile([P, BLK, SW], F32, tag="sp", name=f"sp{b}")
                for tt in range(BLK):
                    for jc in range(2):
                        nc.tensor.matmul(
                            sp[:, tt, :],
                            lhsT=th_t[b][:, jc, tt * P:(tt + 1) * P],
                            rhs=w2sb[:, jc, :],
                            start=(jc == 0),
                            stop=(jc == 1),
                            skip_group_check=True,
                        )
                c0 = b * BLK
                nc.scalar.activation(evec[:, c0:c0 + BLK], sp[:, :, 0:1], Exp)
                del th_t[b]

            # ---- stage D: mask multiply for block g-3 ----
            b = g - 3
            if 0 <= b < nblk:
                c0 = b * BLK
                mk_t[b] = mkp.tile([P, P, BLK], BF16, tag="mk", name=f"mk{b}")
                ev_b = evec[:, c0:c0 + BLK].unsqueeze(1).broadcast_to(
                    [P, P, BLK])
                nc.vector.tensor_tensor(
                    mk_t[b][:], oh_t[b][:], ev_b, op=mybir.AluOpType.mult)
                del oh_t[b]

        out_sb = outp.tile([P, D], F32)
        nc.vector.tensor_copy(out_sb[:], acc)
        nc.sync.dma_start(pooled_d[:], out_sb[:])
        nc.scalar.dma_start(evec_d[:], evec[:])

    nc.compile()
    return nc


def _prep_inputs(x, batch, W1, b1, W2):
    """Shard rows at graph boundaries; build bf16 layouts."""
    x16 = np.asarray(x, dtype=np.float32).astype(BF)
    batch = np.asarray(batch)
    bounds = np.searchsorted(batch, np.arange(0, NUM_GRAPHS + 1, GPC))
    counts = np.diff(bounds)
    chunk = ST * P
    R = int(np.ceil(max(int(counts.max()), 1) / chunk) * chunk)
    T = R // P
    nsup = T // ST

    b1h = np.asarray(b1, dtype=np.float32).reshape(-1)
    with_b1 = bool(np.any(b1h))
    w1h = np.ascontiguousarray(
        np.asarray(W1, dtype=np.float32).reshape(2, P, 2, P).transpose(1, 0, 2, 3)
    ).astype(BF).reshape(P, 2 * 2 * P)
    w2h = np.repeat(
        np.asarray(W2, dtype=np.float32).reshape(2, P).transpose(1, 0)[:, :, None],
        16, axis=2,
    ).astype(BF).reshape(P, 2 * 16)
    b1_pt = np.ascontiguousarray(b1h.reshape(2, P).transpose(1, 0))

    in_maps = []
    for c in range(NC):
        lo, hi = int(bounds[c]), int(bounds[c + 1])
        n = hi - lo
        xs = np.zeros((R, D), dtype=BF)
        xs[:n] = x16[lo:hi]
        x5 = xs.reshape(nsup, ST, P, 2, P)
        # xrow[s*128+p, (t, d)] = x[s*2048+t*128+p, d]
        xr_h = np.ascontiguousarray(
            x5.transpose(0, 2, 1, 3, 4)
        ).reshape(nsup * P, ST * D)
        # xT[s*128+d_lo, (dc, t, i_lo)] = x[s*2048+t*128+i, dc*128+d_lo]
        xt_h = np.ascontiguousarray(
            x5.transpose(0, 4, 3, 1, 2)
        ).reshape(nsup * P, 2 * ST * P)

        br = np.full((R,), -1.0, dtype=np.float32)
        br[:n] = (np.asarray(batch[lo:hi], dtype=np.int64) - c * GPC).astype(
            np.float32)
        brel_pt = np.ascontiguousarray(
            br.reshape(T, P).transpose(1, 0)).astype(BF)
        m = {"xrow": xr_h, "xt": xt_h, "w1": w1h, "w2": w2h, "brel": brel_pt}
        if with_b1:
            m["b1d"] = b1_pt
        in_maps.append(m)
    return in_maps, R, T, with_b1, [int(c) for c in counts]


def run(x, batch, W1, b1, W2, b2, trace=False, trace_kwargs=None):
    in_maps, R, T, with_b1, counts = _prep_inputs(x, batch, W1, b1, W2)
    nc = build_program(R, T, with_b1)
    res = run_bass_kernel_spmd(
        nc, in_maps, core_ids=list(range(NC)), trace=trace,
        **(trace_kwargs or {}),
    )
    pooled = np.zeros((NUM_GRAPHS, D), dtype=np.float64)
    Z = 0.0
    for c in range(NC):
        pooled[c * GPC:(c + 1) * GPC, :] = (
            res.results[c]["pooled"].astype(np.float64))
        ev = res.results[c]["evec_out"].astype(np.float64)  # [P, T] bf16
        n = counts[c]
        rows = ev.transpose(1, 0).reshape(-1)  # row r = t*128+p order
        Z += rows[:n].sum()
    out = (pooled / Z).astype(np.float32)
    return out, res


def kernel(x, batch, W1, b1, W2, b2):
    out, _ = run(x, batch, W1, b1, W2, b2)
    return out
